# revision 28
# baseline (speedup 1.0000x reference)
"""Trainium2 Bass kernel for CGL contrastive region loss.

Problem: proj (96, 256, 64, 64) f32 = 3 stacked views of B=32 images.
Only views 2 and 3 (aug1/aug2) are used. From each image, 25 regions
(5x5 grid of 2x2 windows at centres {10..50}) are extracted over all 256
channels -> region vectors of D = 256*2*2 = 1024. Per image pair the
loss reduces to: for each row r of the 50x50 Gram matrix G of the
stacked normalized regions [u1;u2] (scaled by 1/TAU), LSE over the full
row excluding only the main diagonal entry, minus the positive logit
pos_r = S[r, (r+25)%50]. Data-parallel over batch (4 pairs/core, 8
cores), scalar partials summed on the host.

Device pipeline per core (all 4 pairs batched in 50x200 tiles):
  bf16 inputs, 4 gram chains (8 matmuls each, K=128) into ONE 50x200
  PSUM tile.  diag: one identity mul + one blocked reduce -> d [50,4].
  inv = sqrt(10)*rsqrt(d) = exp(-0.5*ln(d/10)) on ACT.
  G+mask -> SBUF (mask = -1e30 on each block diagonal: exact row-LSE
  diagonal removal; doubles as the PSUM->SBUF move).
  colscale broadcast binv4 = ones^T @ diag(inv) (one f32r PE matmul),
  S = (G+mask) * binv4 * inv_row (two DVE muls), eall = exp(S - 10)
  (one ACT op, valid LSE shift since diag(S)=10), esum = blocked row
  reduce.  lse-10 = ln(esum) (ACT).  pos: -2*pos via a -2*I25 constant
  mul + reduce, summed together with ln(esum) in one 50x8 reduce.
  total = partition-sum matmul; scale + (+10 shift restored) constant
  folded into one tensor_scalar; result leaves via reg_load + posted
  TENSOR_STORE (no output DMA ring round-trip).

ACT tables: every activation (Ln, Exp) is served by the single function
set `natural_log_exp_and_others`, forced by pointing both bacc's
insert_act_table_loads and walrus (BASS_ACT_ROOT_JSON_PATH) at a
patched act_info.json in which no other set contains exp/ln. One table
load, pulled to the head of the ACT queue by a dummy activation and
hidden under the input DMAs. (The default greedy assignment alternates
natural_log/exp_and_others sets, reloading a 1.3us table at every
Ln<->Exp transition, several on the critical path.)

Span overheads trimmed: Bass-init const memsets + entry all-engine
barrier deleted from the BIR (the NRT preamble already runs two
all-engine rendezvous and no const APs are referenced), so the input
DMA triggers issue right after the NRT preamble. Tile tail uses a
sem-only drain barrier. The NRT preamble (~5.5-7us) and postamble
semaphore wipe (~6.5us) are runtime-injected and immovable.
"""

import os
import numpy as np

NB = 4                    # pairs per core
NCORES = 8
R = 25
FREE = NB * 8 * 50        # 1600 free elements per core
_CENTRES = (10, 20, 30, 40, 50)

# cf layout (f32r bits, mostly consumed through an f32 bitcast view;
# per-pair block constants are free-dim stride-0 broadcasts of one copy):
#   [0:50)    ident: 50x50 identity
#   [50:100)  mask: -1e30 on the diagonal, 0 elsewhere
#   [100:150) ones 50x50 (f32r stationary for the colscale matmul;
#             col 100 doubles as the ones column for the final sum)
#   [150:175) negident25: rows 0:25 = -2 * I25 (positive-logit
#             extraction, the -2 loss weight pre-folded)
#   [175] -10.0   [176] 0.0
_CF_COLS = 177

_nc_cache = None


def _build_consts():
    cf = np.zeros((50, _CF_COLS), dtype=np.float32)
    cf[:, 0:50] = np.eye(50)
    cf[:, 50:100] = np.eye(50) * np.float32(-1e30)
    cf[:, 100:150] = 1.0
    cf[0:25, 150:175] = np.eye(25) * np.float32(-2.0)
    cf[:, 175] = -10.0
    cf[:, 176] = 0.0
    return cf


def _patched_act_root():
    """Stage a copy of the neuronxcc pwp table dir whose act_info.json
    leaves `natural_log_exp_and_others` as the only set containing exp or
    ln, so every activation resolves to one table set (single load)."""
    import json
    import shutil
    import tempfile

    import neuronxcc

    src = os.path.join(os.path.dirname(neuronxcc.__file__), "pwp", "pwp_bin_trainium")
    dst = os.path.join(tempfile.gettempdir(), "pwp_nlexp_%d" % os.getuid())
    marker = os.path.join(dst, ".patched_ok")
    if not os.path.exists(marker):
        if os.path.exists(dst):
            shutil.rmtree(dst)
        shutil.copytree(src, dst)
        p = os.path.join(dst, "act_info.json")
        os.chmod(p, 0o644)
        with open(p) as f:
            d = json.load(f)
        for e in d["act_func_sets"]:
            if e["name"] != "natural_log_exp_and_others":
                e["act"].pop("exp", None)
                e["act"].pop("ln", None)
        with open(p, "w") as f:
            json.dump(d, f)
        with open(marker, "w") as f:
            f.write("ok")
    return os.path.join(dst, "act_info.json")


def _apply_act_surgery():
    import functools
    import json

    import concourse.bacc as baccmod

    act_json = _patched_act_root()
    os.environ["BASS_ACT_ROOT_JSON_PATH"] = act_json

    @functools.cache
    def patched_tables(arch):
        from concourse import mybir

        with open(act_json) as f:
            d = json.load(f)
        return {
            e["name"]: {
                mybir.ActivationFunctionType.from_pwp(v) for v in e["act"].keys()
            }
            for e in d["act_func_sets"]
        }

    baccmod.get_activation_tables = patched_tables


def _strip_init_overhead(nc):
    """Remove the Bass-init const memsets and entry all-engine barrier from
    the 'main' block. No const APs are referenced by this kernel, and the
    NRT preamble already synchronizes all engines before the program runs."""
    from concourse import mybir

    for func in nc.m.functions:
        for blk in func.blocks:
            if blk.name != "main":
                continue
            kept = []
            for inst in blk.instructions:
                if isinstance(
                    inst,
                    (mybir.InstMemset, mybir.InstDrain, mybir.InstEventSemaphore),
                ):
                    continue
                kept.append(inst)
            blk.instructions[:] = kept


def _build_nc():
    _apply_act_surgery()

    import concourse.bacc as bacc
    import concourse.tile as tile
    from concourse import mybir
    from concourse.vector_clock import ScopedClock

    class FastTailTileContext(tile.TileContext):
        """Tile tail without the two full all-engine barriers.

        The sync-engine drain already waits on the global vector clock
        (every instruction's sem tick), so once it completes nothing is
        in flight; a sem-only EVSEM barrier then orders the gpsimd
        sem_clears after it."""

        def _drain_and_barrier(self, tick_clock, wait_clock):
            drain_inst = self.nc.sync.drain()
            wait_clock.add_sem_waits(
                drain_inst.ins, ScopedClock({None: tick_clock.global_clock})
            )
            self.nc.all_engine_barrier(sem_only=True)
            popped = self.nc._tile_sem_poison_stack.pop()
            assert popped is self._sem_poison
            self.nc.clear_and_free_semaphores(list(self.sems.allocated().values()))

    f32 = mybir.dt.float32
    f32r = mybir.dt.float32r
    bf16 = mybir.dt.bfloat16
    i32 = mybir.dt.int32
    Alu = mybir.AluOpType
    Act = mybir.ActivationFunctionType
    X = mybir.AxisListType.X

    nc = bacc.Bacc("TRN2", target_bir_lowering=False, debug=False)
    u_dram = nc.dram_tensor("u", [128, FREE], bf16, kind="ExternalInput").ap()
    cf_dram = nc.dram_tensor("cf", [50, _CF_COLS], f32r, kind="ExternalInput").ap()
    out_dram = nc.dram_tensor("out", [1, 1], f32, kind="ExternalOutput").ap()

    def blk(ap, f=50):
        return ap.rearrange("p (b f) -> p b f", f=f)

    # raw (non-tile) SBUF scalar for the final result so the post-tile
    # reg_load sees a concrete (non-symbolic) access pattern
    res_t = nc.alloc_sbuf_tensor("res_scalar", [1, 1], f32)

    with FastTailTileContext(nc) as tc:
        with (
            tc.tile_pool(name="data", bufs=1) as data,
            tc.tile_pool(name="consts", bufs=1) as consts,
            tc.tile_pool(name="work", bufs=2) as work,
            tc.tile_pool(name="psg", bufs=1, space="PSUM") as psg,
            tc.tile_pool(name="psb", bufs=1, space="PSUM") as psb,
            tc.tile_pool(name="pst", bufs=1, space="PSUM") as pst,
        ):
            H = FREE // 2
            # u halves (1600B rows keep the DMA rings at full burst
            # efficiency). Each half is additionally split by partition
            # across BOTH HWDGE rings so the two transfers of a half run
            # concurrently: pairs 0-1 land in ~0.55us instead of ~1.05.
            ubs = []
            cf = consts.tile([50, _CF_COLS], f32r)
            for h in range(2):
                ubh = data.tile([128, H], bf16, tag=f"ub{h}")
                cols = u_dram[:, h * H : (h + 1) * H]
                nc.sync.dma_start(ubh[0:64, :], cols[0:64, :])
                nc.scalar.dma_start(ubh[64:128, :], cols[64:128, :])
                ubs.append(ubh)
            nc.sync.dma_start(cf[:], cf_dram)

            cff = cf[:].bitcast(f32)
            ident = cff[:, 0:50]
            mask = cff[:, 50:100]
            ones50r = cf[:, 100:150]
            ones_col = cff[:, 100:101]
            negident = cff[0:25, 150:175]
            b_m10 = cff[:, 175:176]
            b_zero = cff[:, 176:177]
            identB = ident.unsqueeze(1).broadcast_to([50, NB, 50])
            maskB = mask.unsqueeze(1).broadcast_to([50, NB, 50])
            negidentB = negident.unsqueeze(1).broadcast_to([25, NB, 25])

            # dummy activation on a memset scratch (no DMA deps): pulls the
            # single ACT table load to the head of the ACT queue, fully
            # hidden under the input DMAs
            tscr = work.tile([1, 1], f32, tag="tscr")
            nc.vector.memset(tscr[:], 1.0)
            nc.scalar.activation(tscr[:], tscr[:], Act.Exp, bias=tscr[:])

            # zero rows 25:50 of the -2*pos half of the final-sum tile
            fin = work.tile([50, 2 * NB], f32, tag="fin")
            nc.vector.memset(fin[:], 0.0)

            # 4 gram chains into one 50x200 PSUM tile
            gpall = psg.tile([50, 200], f32, tag="g")
            for b in range(NB):
                gslice = gpall[:, b * 50 : (b + 1) * 50]
                base = (b % 2) * 400
                for k in range(8):
                    sl = ubs[b // 2][:, base + k * 50 : base + (k + 1) * 50]
                    nc.tensor.matmul(gslice, sl, sl, start=(k == 0), stop=(k == 7))

            # block diagonals -> squared norms d [50,4]
            dmul = work.tile([50, 200], f32, tag="dmul")
            nc.vector.tensor_mul(blk(dmul[:]), blk(gpall[:]), identB)
            dsq = work.tile([50, NB], f32, tag="dsq")
            nc.vector.reduce_sum(dsq[:], blk(dmul[:]), axis=X)

            # inv = sqrt(10)*rsqrt(d) = exp(-0.5*ln(d/10)) on ACT
            lnd = work.tile([50, NB], f32, tag="lnd")
            nc.scalar.activation(lnd[:], dsq[:], Act.Ln, bias=b_zero, scale=0.1)
            inv = work.tile([50, NB], f32, tag="inv")
            nc.scalar.activation(inv[:], lnd[:], Act.Exp, bias=b_zero, scale=-0.5)

            # PSUM -> SBUF move doubling as the exact diagonal kill
            gsb = work.tile([50, 200], f32, tag="gsb")
            nc.vector.tensor_add(blk(gsb[:]), blk(gpall[:]), maskB)

            # colscale row: binv4[p,(b,c)] = inv[c,b] via ones^T @ diag(inv)
            invrep = inv[:].unsqueeze(2).broadcast_to([50, NB, 50])
            dinv = work.tile([50, 200], f32r, tag="dinv")
            nc.vector.tensor_mul(blk(dinv[:]), identB, invrep)
            binv4 = psb.tile([50, 200], f32, tag="binv4")
            nc.tensor.matmul(binv4[:], ones50r, dinv[:], start=True, stop=True)

            # S = (G+mask) * colscale * rowscale; eall = exp(S-10)
            gcol = work.tile([50, 200], f32, tag="gcol")
            nc.vector.tensor_mul(gcol[:], gsb[:], binv4[:])
            rowsc = work.tile([50, 200], f32, tag="rowsc")
            nc.vector.tensor_mul(blk(rowsc[:]), blk(gcol[:]), invrep)
            eall = work.tile([50, 200], f32, tag="eall")
            nc.scalar.activation(eall[:], rowsc[:], Act.Exp, bias=b_m10)
            esum = work.tile([50, NB], f32, tag="esum")
            nc.vector.reduce_sum(esum[:], blk(eall[:]), axis=X)

            # -2 * positives from the S12/S21 block diagonals of S
            pmul = work.tile([25, NB * 25], f32, tag="pmul")
            nc.vector.tensor_mul(
                blk(pmul[:], f=25),
                blk(rowsc[0:25, :])[:, :, 25:50],
                negidentB,
            )
            nc.vector.reduce_sum(fin[0:25, NB : 2 * NB], blk(pmul[:], f=25), axis=X)

            # lse-10 = ln(esum); one 50x8 reduce sums lse and -2*pos rows
            nc.scalar.activation(fin[:, 0:NB], esum[:], Act.Ln, bias=b_zero)
            acc = work.tile([50, 1], f32, tag="acc")
            nc.vector.reduce_sum(acc[:], fin[:], axis=X)

            # partition sum -> scalar; restore the +10 LSE shift (10*50*NB)
            # and apply 1/(2*R*B) in the same op
            tp = pst.tile([1, 1], f32, tag="tot")
            nc.tensor.matmul(tp[:], acc[:], ones_col, start=True, stop=True)
            scale = 1.0 / (2.0 * R * NB * NCORES)
            nc.vector.tensor_scalar(
                res_t.ap(), tp[:], scale, (10.0 * 50 * NB) * scale,
                op0=Alu.mult, op1=Alu.add,
            )

            # posted TENSOR_STORE of the scalar result: no output DMA
            # round-trip, and the store flight is covered by the tile
            # drain barrier. (registers are untyped 32-bit — move the f32
            # bits via an i32 view)
            with nc.gpsimd.register("resreg") as rreg:
                nc.gpsimd.reg_load(rreg, res_t.ap().bitcast(i32))
                nc.gpsimd.reg_save(out_dram.bitcast(i32), rreg)

    _strip_init_overhead(nc)
    nc.compile()
    return nc


def get_nc():
    global _nc_cache
    if _nc_cache is None:
        _nc_cache = _build_nc()
    return _nc_cache


def pack_inputs(proj: np.ndarray) -> np.ndarray:
    """(96,256,64,64) -> (128, 32, 8, 50) bf16: partition=c%128,
    free=(pair, chunk k=(cb,dy,dx), view, region rh*5+rw)."""
    import ml_dtypes

    win = np.array([[c - 1, c] for c in _CENTRES])  # (5, 2): rows/cols of each window
    v = np.stack([proj[32:64], proj[64:96]], axis=1)  # (32, 2, 256, 64, 64)
    g = v[:, :, :, win[:, :, None, None], win[None, None, :, :]]  # (32,2,256,5,2,5,2)
    g = g.reshape(32, 2, 2, 128, 5, 2, 5, 2)  # b, view, cb, c', rh, dy, rw, dx
    arr = np.transpose(g, (3, 0, 2, 5, 7, 1, 4, 6))  # c', b, cb, dy, dx, view, rh, rw
    return np.ascontiguousarray(arr).reshape(128, 32, 8, 50).astype(ml_dtypes.bfloat16)


def kernel(proj: np.ndarray) -> np.ndarray:
    from concourse.bass_utils import run_bass_kernel_spmd

    nc = get_nc()
    arr = pack_inputs(np.asarray(proj))
    cf = _build_consts()
    in_maps = [
        {
            "u": np.ascontiguousarray(arr[:, c * NB : (c + 1) * NB]).reshape(128, FREE),
            "cf": cf,
        }
        for c in range(NCORES)
    ]
    results = run_bass_kernel_spmd(nc, in_maps, list(range(NCORES))).results
    total = 0.0
    for r in results:
        total += float(r["out"][0, 0])
    return np.float32(total)


# revision 30
# speedup vs baseline: 1.0107x; 1.0107x over previous
"""Trainium2 Bass kernel for CGL contrastive region loss.

Problem: proj (96, 256, 64, 64) f32 = 3 stacked views of B=32 images.
Only views 2 and 3 (aug1/aug2) are used. From each image, 25 regions
(5x5 grid of 2x2 windows at centres {10..50}) are extracted over all 256
channels -> region vectors of D = 256*2*2 = 1024. Per image pair the
loss reduces to: for each row r of the 50x50 Gram matrix G of the
stacked normalized regions [u1;u2] (scaled by 1/TAU), LSE over the full
row excluding only the main diagonal entry, minus the positive logit
pos_r = S[r, (r+25)%50]. Data-parallel over batch (4 pairs/core, 8
cores), scalar partials summed on the host.

Device pipeline per core (all 4 pairs batched in 50x200 tiles):
  bf16 inputs, 4 gram chains (8 matmuls each, K=128) into ONE 50x200
  PSUM tile.  diag: one identity mul + one blocked reduce -> d [50,4].
  inv = sqrt(10)*rsqrt(d) = exp(-0.5*ln(d/10)) on ACT.
  G+mask -> SBUF (mask = -1e30 on each block diagonal: exact row-LSE
  diagonal removal; doubles as the PSUM->SBUF move).
  colscale broadcast binv4 = ones^T @ diag(inv) (one f32r PE matmul),
  S = (G+mask) * binv4 * inv_row (two DVE muls), eall = exp(S - 10)
  (one ACT op, valid LSE shift since diag(S)=10), esum = blocked row
  reduce.  lse-10 = ln(esum) (ACT).  pos: -2*pos via a -2*I25 constant
  mul + reduce, summed together with ln(esum) in one 50x8 reduce.
  total = partition-sum matmul; scale + (+10 shift restored) constant
  folded into one tensor_scalar; result leaves via reg_load + posted
  TENSOR_STORE (no output DMA ring round-trip).

ACT tables: every activation (Ln, Exp) is served by the single function
set `natural_log_exp_and_others`, forced by pointing both bacc's
insert_act_table_loads and walrus (BASS_ACT_ROOT_JSON_PATH) at a
patched act_info.json in which no other set contains exp/ln. One table
load, pulled to the head of the ACT queue by a dummy activation and
hidden under the input DMAs. (The default greedy assignment alternates
natural_log/exp_and_others sets, reloading a 1.3us table at every
Ln<->Exp transition, several on the critical path.)

Span overheads trimmed: Bass-init const memsets + entry all-engine
barrier deleted from the BIR (the NRT preamble already runs two
all-engine rendezvous and no const APs are referenced), so the input
DMA triggers issue right after the NRT preamble. Tile tail uses a
sem-only drain barrier. The NRT preamble (~5.5-7us) and postamble
semaphore wipe (~6.5us) are runtime-injected and immovable.
"""

import os
import numpy as np

NB = 4                    # pairs per core
NCORES = 8
R = 25
FREE = NB * 8 * 50        # 1600 free elements per core
_CENTRES = (10, 20, 30, 40, 50)

# cf layout (f32r bits, mostly consumed through an f32 bitcast view;
# per-pair block constants are free-dim stride-0 broadcasts of one copy):
#   [0:50)    ident: 50x50 identity
#   [50:100)  mask: -1e30 on the diagonal, 0 elsewhere
#   [100:150) ones 50x50 (f32r stationary for the colscale matmul;
#             col 100 doubles as the ones column for the final sum)
#   [150:175) negident25: rows 0:25 = -2 * I25 (positive-logit
#             extraction, the -2 loss weight pre-folded)
#   [175] -10.0   [176] 0.0
_CF_COLS = 177

_nc_cache = None


def _build_consts():
    cf = np.zeros((50, _CF_COLS), dtype=np.float32)
    cf[:, 0:50] = np.eye(50)
    cf[:, 50:100] = np.eye(50) * np.float32(-1e30)
    cf[:, 100:150] = 1.0
    cf[0:25, 150:175] = np.eye(25) * np.float32(-2.0)
    cf[:, 175] = -10.0
    cf[:, 176] = 0.0
    return cf


def _patched_act_root():
    """Stage a copy of the neuronxcc pwp table dir whose act_info.json
    leaves `natural_log_exp_and_others` as the only set containing exp or
    ln, so every activation resolves to one table set (single load)."""
    import json
    import shutil
    import tempfile

    import neuronxcc

    src = os.path.join(os.path.dirname(neuronxcc.__file__), "pwp", "pwp_bin_trainium")
    dst = os.path.join(tempfile.gettempdir(), "pwp_nlexp_%d" % os.getuid())
    marker = os.path.join(dst, ".patched_ok")
    if not os.path.exists(marker):
        if os.path.exists(dst):
            shutil.rmtree(dst)
        shutil.copytree(src, dst)
        p = os.path.join(dst, "act_info.json")
        os.chmod(p, 0o644)
        with open(p) as f:
            d = json.load(f)
        for e in d["act_func_sets"]:
            if e["name"] != "natural_log_exp_and_others":
                e["act"].pop("exp", None)
                e["act"].pop("ln", None)
        with open(p, "w") as f:
            json.dump(d, f)
        with open(marker, "w") as f:
            f.write("ok")
    return os.path.join(dst, "act_info.json")


def _apply_act_surgery():
    import functools
    import json

    import concourse.bacc as baccmod

    act_json = _patched_act_root()
    os.environ["BASS_ACT_ROOT_JSON_PATH"] = act_json

    @functools.cache
    def patched_tables(arch):
        from concourse import mybir

        with open(act_json) as f:
            d = json.load(f)
        return {
            e["name"]: {
                mybir.ActivationFunctionType.from_pwp(v) for v in e["act"].keys()
            }
            for e in d["act_func_sets"]
        }

    baccmod.get_activation_tables = patched_tables


def _strip_init_overhead(nc):
    """Remove the Bass-init const memsets and entry all-engine barrier from
    the 'main' block. No const APs are referenced by this kernel, and the
    NRT preamble already synchronizes all engines before the program runs."""
    from concourse import mybir

    for func in nc.m.functions:
        for blk in func.blocks:
            if blk.name != "main":
                continue
            kept = []
            for inst in blk.instructions:
                if isinstance(
                    inst,
                    (mybir.InstMemset, mybir.InstDrain, mybir.InstEventSemaphore),
                ):
                    continue
                kept.append(inst)
            blk.instructions[:] = kept


def _build_nc():
    _apply_act_surgery()

    import concourse.bacc as bacc
    import concourse.tile as tile
    from concourse import mybir
    from concourse.vector_clock import ScopedClock

    class FastTailTileContext(tile.TileContext):
        """Tile tail without the two full all-engine barriers.

        The sync-engine drain already waits on the global vector clock
        (every instruction's sem tick), so once it completes nothing is
        in flight; a sem-only EVSEM barrier then orders the gpsimd
        sem_clears after it."""

        def _drain_and_barrier(self, tick_clock, wait_clock):
            drain_inst = self.nc.sync.drain()
            wait_clock.add_sem_waits(
                drain_inst.ins, ScopedClock({None: tick_clock.global_clock})
            )
            self.nc.all_engine_barrier(sem_only=True)
            popped = self.nc._tile_sem_poison_stack.pop()
            assert popped is self._sem_poison
            self.nc.clear_and_free_semaphores(list(self.sems.allocated().values()))

    f32 = mybir.dt.float32
    f32r = mybir.dt.float32r
    bf16 = mybir.dt.bfloat16
    i32 = mybir.dt.int32
    Alu = mybir.AluOpType
    Act = mybir.ActivationFunctionType
    X = mybir.AxisListType.X

    nc = bacc.Bacc("TRN2", target_bir_lowering=False, debug=False)
    u_dram = nc.dram_tensor("u", [128, FREE], bf16, kind="ExternalInput").ap()
    cf_dram = nc.dram_tensor("cf", [50, _CF_COLS], f32r, kind="ExternalInput").ap()
    out_dram = nc.dram_tensor("out", [1, 1], f32, kind="ExternalOutput").ap()

    def blk(ap, f=50):
        return ap.rearrange("p (b f) -> p b f", f=f)

    # raw (non-tile) SBUF scalar for the final result so the post-tile
    # reg_load sees a concrete (non-symbolic) access pattern
    res_t = nc.alloc_sbuf_tensor("res_scalar", [1, 1], f32)

    with FastTailTileContext(nc) as tc:
        with (
            tc.tile_pool(name="data", bufs=1) as data,
            tc.tile_pool(name="consts", bufs=1) as consts,
            tc.tile_pool(name="work", bufs=2) as work,
            tc.tile_pool(name="psg", bufs=1, space="PSUM") as psg,
            tc.tile_pool(name="psb", bufs=1, space="PSUM") as psb,
            tc.tile_pool(name="pst", bufs=1, space="PSUM") as pst,
        ):
            H = FREE // 2
            # u halves (1600B rows keep the DMA rings at full burst
            # efficiency): pairs 0-1 on the sync HWDGE ring, 2-3 on
            # scalar; the small const tensor follows on the sync ring
            # (a partition-split across both rings was tried and is a
            # wash: 64-partition transfers halve per-ring throughput)
            ubs = []
            cf = consts.tile([50, _CF_COLS], f32r)
            for h in range(2):
                ubh = data.tile([128, H], bf16, tag=f"ub{h}")
                eng = nc.sync if h == 0 else nc.scalar
                eng.dma_start(ubh[:], u_dram[:, h * H : (h + 1) * H])
                ubs.append(ubh)
            nc.sync.dma_start(cf[:], cf_dram)

            cff = cf[:].bitcast(f32)
            ident = cff[:, 0:50]
            mask = cff[:, 50:100]
            ones50r = cf[:, 100:150]
            ones_col = cff[:, 100:101]
            negident = cff[0:25, 150:175]
            b_m10 = cff[:, 175:176]
            b_zero = cff[:, 176:177]
            identB = ident.unsqueeze(1).broadcast_to([50, NB, 50])
            maskB = mask.unsqueeze(1).broadcast_to([50, NB, 50])
            negidentB = negident.unsqueeze(1).broadcast_to([25, NB, 25])

            # dummy activation on a memset scratch (no DMA deps): pulls the
            # single ACT table load to the head of the ACT queue, fully
            # hidden under the input DMAs
            tscr = work.tile([1, 1], f32, tag="tscr")
            nc.vector.memset(tscr[:], 1.0)
            nc.scalar.activation(tscr[:], tscr[:], Act.Exp, bias=tscr[:])

            # PE p-state warmup: sustained dummy matmuls during the input
            # DMA window so the real gram chains run at full clock
            wscr = work.tile([128, 400], bf16, tag="wscr")
            nc.vector.memset(wscr[:], 0.0)
            psw = pst.tile([50, 400], f32, tag="warm")
            for w in range(4):
                nc.tensor.matmul(
                    psw[:], wscr[:, 0:50], wscr[:],
                    start=(w == 0), stop=(w == 3),
                )

            # zero rows 25:50 of the -2*pos half of the final-sum tile
            fin = work.tile([50, 2 * NB], f32, tag="fin")
            nc.vector.memset(fin[:], 0.0)

            # 4 gram chains into one 50x200 PSUM tile
            gpall = psg.tile([50, 200], f32, tag="g")
            for b in range(NB):
                gslice = gpall[:, b * 50 : (b + 1) * 50]
                base = (b % 2) * 400
                for k in range(8):
                    sl = ubs[b // 2][:, base + k * 50 : base + (k + 1) * 50]
                    nc.tensor.matmul(gslice, sl, sl, start=(k == 0), stop=(k == 7))

            # block diagonals -> squared norms d [50,4]
            dmul = work.tile([50, 200], f32, tag="dmul")
            nc.vector.tensor_mul(blk(dmul[:]), blk(gpall[:]), identB)
            dsq = work.tile([50, NB], f32, tag="dsq")
            nc.vector.reduce_sum(dsq[:], blk(dmul[:]), axis=X)

            # inv = sqrt(10)*rsqrt(d) = exp(-0.5*ln(d/10)) on ACT
            lnd = work.tile([50, NB], f32, tag="lnd")
            nc.scalar.activation(lnd[:], dsq[:], Act.Ln, bias=b_zero, scale=0.1)
            inv = work.tile([50, NB], f32, tag="inv")
            nc.scalar.activation(inv[:], lnd[:], Act.Exp, bias=b_zero, scale=-0.5)

            # PSUM -> SBUF move doubling as the exact diagonal kill
            gsb = work.tile([50, 200], f32, tag="gsb")
            nc.vector.tensor_add(blk(gsb[:]), blk(gpall[:]), maskB)

            # colscale row: binv4[p,(b,c)] = inv[c,b] via ones^T @ diag(inv)
            invrep = inv[:].unsqueeze(2).broadcast_to([50, NB, 50])
            dinv = work.tile([50, 200], f32r, tag="dinv")
            nc.vector.tensor_mul(blk(dinv[:]), identB, invrep)
            binv4 = psb.tile([50, 200], f32, tag="binv4")
            nc.tensor.matmul(binv4[:], ones50r, dinv[:], start=True, stop=True)

            # S = (G+mask) * colscale * rowscale; eall = exp(S-10)
            gcol = work.tile([50, 200], f32, tag="gcol")
            nc.vector.tensor_mul(gcol[:], gsb[:], binv4[:])
            rowsc = work.tile([50, 200], f32, tag="rowsc")
            nc.vector.tensor_mul(blk(rowsc[:]), blk(gcol[:]), invrep)
            eall = work.tile([50, 200], f32, tag="eall")
            nc.scalar.activation(eall[:], rowsc[:], Act.Exp, bias=b_m10)
            esum = work.tile([50, NB], f32, tag="esum")
            nc.vector.reduce_sum(esum[:], blk(eall[:]), axis=X)

            # -2 * positives from the S12/S21 block diagonals of S
            pmul = work.tile([25, NB * 25], f32, tag="pmul")
            nc.vector.tensor_mul(
                blk(pmul[:], f=25),
                blk(rowsc[0:25, :])[:, :, 25:50],
                negidentB,
            )
            nc.vector.reduce_sum(fin[0:25, NB : 2 * NB], blk(pmul[:], f=25), axis=X)

            # lse-10 = ln(esum); one 50x8 reduce sums lse and -2*pos rows
            nc.scalar.activation(fin[:, 0:NB], esum[:], Act.Ln, bias=b_zero)
            acc = work.tile([50, 1], f32, tag="acc")
            nc.vector.reduce_sum(acc[:], fin[:], axis=X)

            # partition sum -> scalar; restore the +10 LSE shift (10*50*NB)
            # and apply 1/(2*R*B) in the same op
            tp = pst.tile([1, 1], f32, tag="tot")
            nc.tensor.matmul(tp[:], acc[:], ones_col, start=True, stop=True)
            scale = 1.0 / (2.0 * R * NB * NCORES)
            nc.vector.tensor_scalar(
                res_t.ap(), tp[:], scale, (10.0 * 50 * NB) * scale,
                op0=Alu.mult, op1=Alu.add,
            )

            # posted TENSOR_STORE of the scalar result: no output DMA
            # round-trip, and the store flight is covered by the tile
            # drain barrier. (registers are untyped 32-bit — move the f32
            # bits via an i32 view)
            with nc.gpsimd.register("resreg") as rreg:
                nc.gpsimd.reg_load(rreg, res_t.ap().bitcast(i32))
                nc.gpsimd.reg_save(out_dram.bitcast(i32), rreg)

    _strip_init_overhead(nc)
    nc.compile()
    return nc


def get_nc():
    global _nc_cache
    if _nc_cache is None:
        _nc_cache = _build_nc()
    return _nc_cache


def pack_inputs(proj: np.ndarray) -> np.ndarray:
    """(96,256,64,64) -> (128, 32, 8, 50) bf16: partition=c%128,
    free=(pair, chunk k=(cb,dy,dx), view, region rh*5+rw)."""
    import ml_dtypes

    win = np.array([[c - 1, c] for c in _CENTRES])  # (5, 2): rows/cols of each window
    v = np.stack([proj[32:64], proj[64:96]], axis=1)  # (32, 2, 256, 64, 64)
    g = v[:, :, :, win[:, :, None, None], win[None, None, :, :]]  # (32,2,256,5,2,5,2)
    g = g.reshape(32, 2, 2, 128, 5, 2, 5, 2)  # b, view, cb, c', rh, dy, rw, dx
    arr = np.transpose(g, (3, 0, 2, 5, 7, 1, 4, 6))  # c', b, cb, dy, dx, view, rh, rw
    return np.ascontiguousarray(arr).reshape(128, 32, 8, 50).astype(ml_dtypes.bfloat16)


def kernel(proj: np.ndarray) -> np.ndarray:
    from concourse.bass_utils import run_bass_kernel_spmd

    nc = get_nc()
    arr = pack_inputs(np.asarray(proj))
    cf = _build_consts()
    in_maps = [
        {
            "u": np.ascontiguousarray(arr[:, c * NB : (c + 1) * NB]).reshape(128, FREE),
            "cf": cf,
        }
        for c in range(NCORES)
    ]
    results = run_bass_kernel_spmd(nc, in_maps, list(range(NCORES))).results
    total = 0.0
    for r in results:
        total += float(r["out"][0, 0])
    return np.float32(total)


# revision 34
# speedup vs baseline: 1.0199x; 1.0092x over previous
"""Trainium2 Bass kernel for CGL contrastive region loss.

Problem: proj (96, 256, 64, 64) f32 = 3 stacked views of B=32 images.
Only views 2 and 3 (aug1/aug2) are used. From each image, 25 regions
(5x5 grid of 2x2 windows at centres {10..50}) are extracted over all 256
channels -> region vectors of D = 256*2*2 = 1024. Per image pair the
loss reduces to: for each row r of the 50x50 Gram matrix G of the
stacked normalized regions [u1;u2] (scaled by 1/TAU), LSE over the full
row excluding only the main diagonal entry, minus the positive logit
pos_r = S[r, (r+25)%50]. Data-parallel over batch (4 pairs/core, 8
cores), scalar partials summed on the host.

Device pipeline per core (all 4 pairs batched in 50x200 tiles):
  bf16 inputs, 4 gram chains (8 matmuls each, K=128) into ONE 50x200
  PSUM tile.  diag: one identity mul + one blocked reduce -> d [50,4].
  inv = sqrt(10)*rsqrt(d) = exp(-0.5*ln(d/10)) on ACT.
  G+mask -> SBUF (mask = -1e30 on each block diagonal: exact row-LSE
  diagonal removal; doubles as the PSUM->SBUF move).
  colscale broadcast binv4 = ones^T @ diag(inv) (one f32r PE matmul),
  S = (G+mask) * binv4 * inv_row (two DVE muls), eall = exp(S - 10)
  (one ACT op, valid LSE shift since diag(S)=10), esum = blocked row
  reduce.  lse-10 = ln(esum) (ACT).  pos: -2*pos via a -2*I25 constant
  mul + reduce, summed together with ln(esum) in one 50x8 reduce.
  total = partition-sum matmul; scale + (+10 shift restored) constant
  folded into one tensor_scalar; result leaves via reg_load + posted
  TENSOR_STORE (no output DMA ring round-trip).

ACT tables: every activation (Ln, Exp) is served by the single function
set `natural_log_exp_and_others`, forced by pointing both bacc's
insert_act_table_loads and walrus (BASS_ACT_ROOT_JSON_PATH) at a
patched act_info.json in which no other set contains exp/ln. One table
load, pulled to the head of the ACT queue by a dummy activation and
hidden under the input DMAs. (The default greedy assignment alternates
natural_log/exp_and_others sets, reloading a 1.3us table at every
Ln<->Exp transition, several on the critical path.)

Span overheads trimmed: Bass-init const memsets + entry all-engine
barrier deleted from the BIR (the NRT preamble already runs two
all-engine rendezvous and no const APs are referenced), so the input
DMA triggers issue right after the NRT preamble. Tile tail uses a
sem-only drain barrier. The NRT preamble (~5.5-7us) and postamble
semaphore wipe (~6.5us) are runtime-injected and immovable.
"""

import os
import numpy as np

NB = 4                    # pairs per core
NCORES = 8
R = 25
FREE = NB * 8 * 50        # 1600 free elements per core
_CENTRES = (10, 20, 30, 40, 50)

# cf layout (f32r bits, mostly consumed through an f32 bitcast view;
# per-pair block constants are free-dim stride-0 broadcasts of one copy):
#   [0:50)    ident: 50x50 identity
#   [50:100)  mask: -1e30 on the diagonal, 0 elsewhere
#   [100:150) ones 50x50 (f32r stationary for the colscale matmul;
#             col 100 doubles as the ones column for the final sum)
#   [150:175) negident25: rows 0:25 = -2 * I25 (positive-logit
#             extraction, the -2 loss weight pre-folded)
#   [175] -10.0   [176] 0.0
_CF_COLS = 177

_nc_cache = None


def _build_consts():
    cf = np.zeros((50, _CF_COLS), dtype=np.float32)
    cf[:, 0:50] = np.eye(50)
    cf[:, 50:100] = np.eye(50) * np.float32(-1e30)
    cf[:, 100:150] = 1.0
    cf[0:25, 150:175] = np.eye(25) * np.float32(-2.0)
    cf[:, 175] = -10.0
    cf[:, 176] = 0.0
    return cf


def _patched_act_root():
    """Stage a copy of the neuronxcc pwp table dir whose act_info.json
    leaves `natural_log_exp_and_others` as the only set containing exp or
    ln, so every activation resolves to one table set (single load)."""
    import json
    import shutil
    import tempfile

    import neuronxcc

    src = os.path.join(os.path.dirname(neuronxcc.__file__), "pwp", "pwp_bin_trainium")
    dst = os.path.join(tempfile.gettempdir(), "pwp_nlexp_%d" % os.getuid())
    marker = os.path.join(dst, ".patched_ok")
    if not os.path.exists(marker):
        if os.path.exists(dst):
            shutil.rmtree(dst)
        shutil.copytree(src, dst)
        p = os.path.join(dst, "act_info.json")
        os.chmod(p, 0o644)
        with open(p) as f:
            d = json.load(f)
        for e in d["act_func_sets"]:
            if e["name"] != "natural_log_exp_and_others":
                e["act"].pop("exp", None)
                e["act"].pop("ln", None)
        with open(p, "w") as f:
            json.dump(d, f)
        with open(marker, "w") as f:
            f.write("ok")
    return os.path.join(dst, "act_info.json")


def _apply_act_surgery():
    import functools
    import json

    import concourse.bacc as baccmod

    act_json = _patched_act_root()
    os.environ["BASS_ACT_ROOT_JSON_PATH"] = act_json

    @functools.cache
    def patched_tables(arch):
        from concourse import mybir

        with open(act_json) as f:
            d = json.load(f)
        return {
            e["name"]: {
                mybir.ActivationFunctionType.from_pwp(v) for v in e["act"].keys()
            }
            for e in d["act_func_sets"]
        }

    baccmod.get_activation_tables = patched_tables


def _strip_init_overhead(nc):
    """Remove the Bass-init const memsets and entry all-engine barrier from
    the 'main' block. No const APs are referenced by this kernel, and the
    NRT preamble already synchronizes all engines before the program runs."""
    from concourse import mybir

    for func in nc.m.functions:
        for blk in func.blocks:
            if blk.name != "main":
                continue
            kept = []
            for inst in blk.instructions:
                if isinstance(
                    inst,
                    (mybir.InstMemset, mybir.InstDrain, mybir.InstEventSemaphore),
                ):
                    continue
                kept.append(inst)
            blk.instructions[:] = kept


def _build_nc():
    _apply_act_surgery()

    import concourse.bacc as bacc
    import concourse.tile as tile
    from concourse import mybir
    from concourse.vector_clock import ScopedClock

    class FastTailTileContext(tile.TileContext):
        """Tile tail without the two full all-engine barriers.

        The sync-engine drain already waits on the global vector clock
        (every instruction's sem tick), so once it completes nothing is
        in flight; a sem-only EVSEM barrier then orders the gpsimd
        sem_clears after it."""

        def _drain_and_barrier(self, tick_clock, wait_clock):
            drain_inst = self.nc.sync.drain()
            wait_clock.add_sem_waits(
                drain_inst.ins, ScopedClock({None: tick_clock.global_clock})
            )
            self.nc.all_engine_barrier(sem_only=True)
            popped = self.nc._tile_sem_poison_stack.pop()
            assert popped is self._sem_poison
            self.nc.clear_and_free_semaphores(list(self.sems.allocated().values()))

    f32 = mybir.dt.float32
    f32r = mybir.dt.float32r
    bf16 = mybir.dt.bfloat16
    i32 = mybir.dt.int32
    Alu = mybir.AluOpType
    Act = mybir.ActivationFunctionType
    X = mybir.AxisListType.X

    nc = bacc.Bacc("TRN2", target_bir_lowering=False, debug=False)
    u_dram = nc.dram_tensor("u", [128, FREE], bf16, kind="ExternalInput").ap()
    cf_dram = nc.dram_tensor("cf", [50, _CF_COLS], f32r, kind="ExternalInput").ap()
    out_h = nc.dram_tensor("out", [1, 1], f32, kind="ExternalOutput")
    out_dram = out_h.ap()
    # runtime-populated pointer to the output buffer: loading it into a
    # register pair EARLY (under the DMA window) keeps the ~1us DRAM
    # pointer fetch off the critical path of the final store
    out_ptr = nc.pointer_tensor(out_h)

    def blk(ap, f=50):
        return ap.rearrange("p (b f) -> p b f", f=f)

    # raw (non-tile) SBUF scalar for the final result so the post-tile
    # reg_load sees a concrete (non-symbolic) access pattern
    res_t = nc.alloc_sbuf_tensor("res_scalar", [1, 1], f32)

    with FastTailTileContext(nc) as tc:
        with (
            tc.tile_pool(name="data", bufs=1) as data,
            tc.tile_pool(name="consts", bufs=1) as consts,
            tc.tile_pool(name="work", bufs=2) as work,
            tc.tile_pool(name="psg", bufs=1, space="PSUM") as psg,
            tc.tile_pool(name="psb", bufs=1, space="PSUM") as psb,
            tc.tile_pool(name="pst", bufs=1, space="PSUM") as pst,
            nc.gpsimd.register64("outaddr") as opair,
            nc.gpsimd.register("resreg") as rreg,
        ):
            # no deps -> issues at the head of the gpsimd queue
            # (the u64 pointer is loaded as a 2 x i32 register pair)
            nc.gpsimd.load(opair, out_ptr.ap().bitcast(i32))
            H = FREE // 2
            # u halves (1600B rows keep the DMA rings at full burst
            # efficiency): pairs 0-1 on the sync HWDGE ring, 2-3 on
            # scalar; the small const tensor follows on the sync ring
            # (a partition-split across both rings was tried and is a
            # wash: 64-partition transfers halve per-ring throughput)
            ubs = []
            cf = consts.tile([50, _CF_COLS], f32r)
            for h in range(2):
                ubh = data.tile([128, H], bf16, tag=f"ub{h}")
                eng = nc.sync if h == 0 else nc.scalar
                eng.dma_start(ubh[:], u_dram[:, h * H : (h + 1) * H])
                ubs.append(ubh)
            nc.sync.dma_start(cf[:], cf_dram)

            cff = cf[:].bitcast(f32)
            ident = cff[:, 0:50]
            mask = cff[:, 50:100]
            ones50r = cf[:, 100:150]
            ones_col = cff[:, 100:101]
            negident = cff[0:25, 150:175]
            b_m10 = cff[:, 175:176]
            b_zero = cff[:, 176:177]
            identB = ident.unsqueeze(1).broadcast_to([50, NB, 50])
            maskB = mask.unsqueeze(1).broadcast_to([50, NB, 50])
            negidentB = negident.unsqueeze(1).broadcast_to([25, NB, 25])

            # dummy activation on a memset scratch (no DMA deps): pulls the
            # single ACT table load to the head of the ACT queue, fully
            # hidden under the input DMAs
            tscr = work.tile([1, 1], f32, tag="tscr")
            nc.vector.memset(tscr[:], 1.0)
            nc.scalar.activation(tscr[:], tscr[:], Act.Exp, bias=tscr[:])

            # PE p-state warmup: sustained dummy matmuls during the input
            # DMA window so the real gram chains run at full clock
            wscr = work.tile([128, 400], bf16, tag="wscr")
            nc.vector.memset(wscr[:], 0.0)
            psw = pst.tile([50, 400], f32, tag="warm")
            for w in range(4):
                nc.tensor.matmul(
                    psw[:], wscr[:, 0:50], wscr[:],
                    start=(w == 0), stop=(w == 3),
                )

            # zero rows 25:50 of the -2*pos half of the final-sum tile
            fin = work.tile([50, 2 * NB], f32, tag="fin")
            nc.vector.memset(fin[:], 0.0)

            # 4 gram chains into one 50x200 PSUM tile
            gpall = psg.tile([50, 200], f32, tag="g")
            for b in range(NB):
                gslice = gpall[:, b * 50 : (b + 1) * 50]
                base = (b % 2) * 400
                for k in range(8):
                    sl = ubs[b // 2][:, base + k * 50 : base + (k + 1) * 50]
                    nc.tensor.matmul(gslice, sl, sl, start=(k == 0), stop=(k == 7))

            # block diagonals -> squared norms d [50,4]
            dmul = work.tile([50, 200], f32, tag="dmul")
            nc.vector.tensor_mul(blk(dmul[:]), blk(gpall[:]), identB)
            dsq = work.tile([50, NB], f32, tag="dsq")
            nc.vector.reduce_sum(dsq[:], blk(dmul[:]), axis=X)

            # inv = sqrt(10)*rsqrt(d) = exp(-0.5*ln(d/10)) on ACT
            lnd = work.tile([50, NB], f32, tag="lnd")
            nc.scalar.activation(lnd[:], dsq[:], Act.Ln, bias=b_zero, scale=0.1)
            inv = work.tile([50, NB], f32, tag="inv")
            nc.scalar.activation(inv[:], lnd[:], Act.Exp, bias=b_zero, scale=-0.5)

            # PSUM -> SBUF move doubling as the exact diagonal kill
            gsb = work.tile([50, 200], f32, tag="gsb")
            nc.vector.tensor_add(blk(gsb[:]), blk(gpall[:]), maskB)

            # colscale row: binv4[p,(b,c)] = inv[c,b] via ones^T @ diag(inv)
            invrep = inv[:].unsqueeze(2).broadcast_to([50, NB, 50])
            dinv = work.tile([50, 200], f32r, tag="dinv")
            nc.vector.tensor_mul(blk(dinv[:]), identB, invrep)
            binv4 = psb.tile([50, 200], f32, tag="binv4")
            nc.tensor.matmul(binv4[:], ones50r, dinv[:], start=True, stop=True)

            # S = (G+mask) * colscale * rowscale; eall = exp(S-10)
            gcol = work.tile([50, 200], f32, tag="gcol")
            nc.vector.tensor_mul(gcol[:], gsb[:], binv4[:])
            rowsc = work.tile([50, 200], f32, tag="rowsc")
            nc.vector.tensor_mul(blk(rowsc[:]), blk(gcol[:]), invrep)
            eall = work.tile([50, 200], f32, tag="eall")
            nc.scalar.activation(eall[:], rowsc[:], Act.Exp, bias=b_m10)
            esum = work.tile([50, NB], f32, tag="esum")
            nc.vector.reduce_sum(esum[:], blk(eall[:]), axis=X)

            # -2 * positives from the S12/S21 block diagonals of S
            pmul = work.tile([25, NB * 25], f32, tag="pmul")
            nc.vector.tensor_mul(
                blk(pmul[:], f=25),
                blk(rowsc[0:25, :])[:, :, 25:50],
                negidentB,
            )
            nc.vector.reduce_sum(fin[0:25, NB : 2 * NB], blk(pmul[:], f=25), axis=X)

            # lse-10 = ln(esum); one 50x8 reduce sums lse and -2*pos rows
            nc.scalar.activation(fin[:, 0:NB], esum[:], Act.Ln, bias=b_zero)
            acc = work.tile([50, 1], f32, tag="acc")
            nc.vector.reduce_sum(acc[:], fin[:], axis=X)

            # partition sum -> scalar; restore the +10 LSE shift (10*50*NB)
            # and apply 1/(2*R*B) in the same op
            tp = pst.tile([1, 1], f32, tag="tot")
            nc.tensor.matmul(tp[:], acc[:], ones_col, start=True, stop=True)
            scale = 1.0 / (2.0 * R * NB * NCORES)
            nc.vector.tensor_scalar(
                res_t.ap(), tp[:], scale, (10.0 * 50 * NB) * scale,
                op0=Alu.mult, op1=Alu.add,
            )

            # posted TENSOR_STORE of the scalar result through the
            # preloaded address pair: no output DMA round-trip, no pointer
            # fetch on the critical path, and the store flight is covered
            # by the tile drain barrier. (registers are untyped 32-bit —
            # move the f32 bits via an i32 view)
            nc.gpsimd.reg_load(rreg, res_t.ap().bitcast(i32))
            nc.gpsimd.store(opair, rreg)

    _strip_init_overhead(nc)
    nc.compile()
    return nc


def get_nc():
    global _nc_cache
    if _nc_cache is None:
        _nc_cache = _build_nc()
    return _nc_cache


def pack_inputs(proj: np.ndarray) -> np.ndarray:
    """(96,256,64,64) -> (128, 32, 8, 50) bf16: partition=c%128,
    free=(pair, chunk k=(cb,dy,dx), view, region rh*5+rw)."""
    import ml_dtypes

    win = np.array([[c - 1, c] for c in _CENTRES])  # (5, 2): rows/cols of each window
    v = np.stack([proj[32:64], proj[64:96]], axis=1)  # (32, 2, 256, 64, 64)
    g = v[:, :, :, win[:, :, None, None], win[None, None, :, :]]  # (32,2,256,5,2,5,2)
    g = g.reshape(32, 2, 2, 128, 5, 2, 5, 2)  # b, view, cb, c', rh, dy, rw, dx
    arr = np.transpose(g, (3, 0, 2, 5, 7, 1, 4, 6))  # c', b, cb, dy, dx, view, rh, rw
    return np.ascontiguousarray(arr).reshape(128, 32, 8, 50).astype(ml_dtypes.bfloat16)


def kernel(proj: np.ndarray) -> np.ndarray:
    from concourse.bass_utils import run_bass_kernel_spmd

    nc = get_nc()
    arr = pack_inputs(np.asarray(proj))
    cf = _build_consts()
    in_maps = [
        {
            "u": np.ascontiguousarray(arr[:, c * NB : (c + 1) * NB]).reshape(128, FREE),
            "cf": cf,
        }
        for c in range(NCORES)
    ]
    results = run_bass_kernel_spmd(nc, in_maps, list(range(NCORES))).results
    total = 0.0
    for r in results:
        total += float(r["out"][0, 0])
    return np.float32(total)


# revision 36
# speedup vs baseline: 1.0494x; 1.0289x over previous
"""Trainium2 Bass kernel for CGL contrastive region loss.

Problem: proj (96, 256, 64, 64) f32 = 3 stacked views of B=32 images.
Only views 2 and 3 (aug1/aug2) are used. From each image, 25 regions
(5x5 grid of 2x2 windows at centres {10..50}) are extracted over all 256
channels -> region vectors of D = 256*2*2 = 1024. Per image pair the
loss reduces to: for each row r of the 50x50 Gram matrix G of the
stacked normalized regions [u1;u2] (scaled by 1/TAU), LSE over the full
row excluding only the main diagonal entry, minus the positive logit
pos_r = S[r, (r+25)%50]. Data-parallel over batch (4 pairs/core, 8
cores), scalar partials summed on the host.

Device pipeline per core (all 4 pairs batched in 50x200 tiles):
  bf16 inputs, 4 gram chains (8 matmuls each, K=128) into ONE 50x200
  PSUM tile.  diag: one identity mul + one blocked reduce -> d [50,4].
  inv = sqrt(10)*rsqrt(d) = exp(-0.5*ln(d/10)) on ACT.
  G+mask -> SBUF (mask = -1e30 on each block diagonal: exact row-LSE
  diagonal removal; doubles as the PSUM->SBUF move).
  colscale broadcast binv4 = ones^T @ diag(inv) (one f32r PE matmul),
  S = (G+mask) * binv4 * inv_row (two DVE muls), eall = exp(S - 10)
  (one ACT op, valid LSE shift since diag(S)=10), esum = blocked row
  reduce.  lse-10 = ln(esum) (ACT).  pos: -2*pos via a -2*I25 constant
  mul + reduce, summed together with ln(esum) in one 50x8 reduce.
  total = partition-sum matmul; scale + (+10 shift restored) constant
  folded into one tensor_scalar; result leaves via reg_load + posted
  TENSOR_STORE (no output DMA ring round-trip).

ACT tables: every activation (Ln, Exp) is served by the single function
set `natural_log_exp_and_others`, forced by pointing both bacc's
insert_act_table_loads and walrus (BASS_ACT_ROOT_JSON_PATH) at a
patched act_info.json in which no other set contains exp/ln. One table
load, pulled to the head of the ACT queue by a dummy activation and
hidden under the input DMAs. (The default greedy assignment alternates
natural_log/exp_and_others sets, reloading a 1.3us table at every
Ln<->Exp transition, several on the critical path.)

Span overheads trimmed: Bass-init const memsets + entry all-engine
barrier deleted from the BIR (the NRT preamble already runs two
all-engine rendezvous and no const APs are referenced), so the input
DMA triggers issue right after the NRT preamble. Tile tail uses a
sem-only drain barrier. The NRT preamble (~5.5-7us) and postamble
semaphore wipe (~6.5us) are runtime-injected and immovable.
"""

import os
import numpy as np

NB = 4                    # pairs per core
NCORES = 8
R = 25
FREE = NB * 8 * 50        # 1600 free elements per core
_CENTRES = (10, 20, 30, 40, 50)

# cf layout (f32r bits, mostly consumed through an f32 bitcast view;
# per-pair block constants are free-dim stride-0 broadcasts of one copy):
#   [0:50)    ident: 50x50 identity
#   [50:100)  mask: -1e30 on the diagonal, 0 elsewhere
#   [100:150) ones 50x50 (f32r stationary for the colscale matmul;
#             col 100 doubles as the ones column for the final sum)
#   [150:175) negident25: rows 0:25 = -2 * I25 (positive-logit
#             extraction, the -2 loss weight pre-folded)
#   [175] -10.0   [176] 0.0
_CF_COLS = 177

_nc_cache = None


def _build_consts():
    cf = np.zeros((50, _CF_COLS), dtype=np.float32)
    cf[:, 0:50] = np.eye(50)
    cf[:, 50:100] = np.eye(50) * np.float32(-1e30)
    cf[:, 100:150] = 1.0
    cf[0:25, 150:175] = np.eye(25) * np.float32(-2.0)
    cf[:, 175] = -10.0
    cf[:, 176] = 0.0
    return cf


def _patched_act_root():
    """Stage a copy of the neuronxcc pwp table dir whose act_info.json
    leaves `natural_log_exp_and_others` as the only set containing exp or
    ln, so every activation resolves to one table set (single load)."""
    import json
    import shutil
    import tempfile

    import neuronxcc

    src = os.path.join(os.path.dirname(neuronxcc.__file__), "pwp", "pwp_bin_trainium")
    dst = os.path.join(tempfile.gettempdir(), "pwp_nlexp_%d" % os.getuid())
    marker = os.path.join(dst, ".patched_ok")
    if not os.path.exists(marker):
        if os.path.exists(dst):
            shutil.rmtree(dst)
        shutil.copytree(src, dst)
        p = os.path.join(dst, "act_info.json")
        os.chmod(p, 0o644)
        with open(p) as f:
            d = json.load(f)
        for e in d["act_func_sets"]:
            if e["name"] != "natural_log_exp_and_others":
                e["act"].pop("exp", None)
                e["act"].pop("ln", None)
        with open(p, "w") as f:
            json.dump(d, f)
        with open(marker, "w") as f:
            f.write("ok")
    return os.path.join(dst, "act_info.json")


def _apply_act_surgery():
    import functools
    import json

    import concourse.bacc as baccmod

    act_json = _patched_act_root()
    os.environ["BASS_ACT_ROOT_JSON_PATH"] = act_json

    @functools.cache
    def patched_tables(arch):
        from concourse import mybir

        with open(act_json) as f:
            d = json.load(f)
        return {
            e["name"]: {
                mybir.ActivationFunctionType.from_pwp(v) for v in e["act"].keys()
            }
            for e in d["act_func_sets"]
        }

    baccmod.get_activation_tables = patched_tables


def _strip_init_overhead(nc):
    """Remove the Bass-init const memsets and entry all-engine barrier from
    the 'main' block. No const APs are referenced by this kernel, and the
    NRT preamble already synchronizes all engines before the program runs."""
    from concourse import mybir

    for func in nc.m.functions:
        for blk in func.blocks:
            if blk.name != "main":
                continue
            kept = []
            for inst in blk.instructions:
                if isinstance(
                    inst,
                    (mybir.InstMemset, mybir.InstDrain, mybir.InstEventSemaphore),
                ):
                    continue
                kept.append(inst)
            blk.instructions[:] = kept


def _build_nc():
    _apply_act_surgery()

    import concourse.bacc as bacc
    import concourse.tile as tile
    from concourse import mybir
    from concourse.vector_clock import ScopedClock

    class FastTailTileContext(tile.TileContext):
        """Tile tail without the two full all-engine barriers.

        The sync-engine drain already waits on the global vector clock
        (every instruction's sem tick), so once it completes nothing is
        in flight; a sem-only EVSEM barrier then orders the gpsimd
        sem_clears after it."""

        def _drain_and_barrier(self, tick_clock, wait_clock):
            drain_inst = self.nc.sync.drain()
            wait_clock.add_sem_waits(
                drain_inst.ins, ScopedClock({None: tick_clock.global_clock})
            )
            self.nc.all_engine_barrier(sem_only=True)
            popped = self.nc._tile_sem_poison_stack.pop()
            assert popped is self._sem_poison
            self.nc.clear_and_free_semaphores(list(self.sems.allocated().values()))

    f32 = mybir.dt.float32
    f32r = mybir.dt.float32r
    bf16 = mybir.dt.bfloat16
    i32 = mybir.dt.int32
    Alu = mybir.AluOpType
    Act = mybir.ActivationFunctionType
    X = mybir.AxisListType.X

    nc = bacc.Bacc("TRN2", target_bir_lowering=False, debug=False)
    u_dram = nc.dram_tensor("u", [128, FREE], bf16, kind="ExternalInput").ap()
    cf_dram = nc.dram_tensor("cf", [50, _CF_COLS], f32r, kind="ExternalInput").ap()
    out_h = nc.dram_tensor("out", [1, 1], f32, kind="ExternalOutput")
    out_dram = out_h.ap()
    # runtime-populated pointer to the output buffer: loading it into a
    # register pair EARLY (under the DMA window) keeps the ~1us DRAM
    # pointer fetch off the critical path of the final store
    out_ptr = nc.pointer_tensor(out_h)

    def blk(ap, f=50):
        return ap.rearrange("p (b f) -> p b f", f=f)

    # raw (non-tile) SBUF scalar for the final result so the post-tile
    # reg_load sees a concrete (non-symbolic) access pattern
    res_t = nc.alloc_sbuf_tensor("res_scalar", [1, 1], f32)

    import contextlib

    _regs = contextlib.ExitStack()
    opair = _regs.enter_context(nc.gpsimd.register64("outaddr"))
    rreg = _regs.enter_context(nc.gpsimd.register("resreg"))
    # pre-TileContext 'main'-block instruction: the gpsimd queue executes it
    # immediately after the NRT preamble, hiding the ~1us DRAM pointer
    # fetch under the input DMA window (the tile scheduler would otherwise
    # sink it next to the final store). u64 pointer = 2 x i32 registers.
    nc.gpsimd.load(opair, out_ptr.ap().bitcast(mybir.dt.int32))

    with FastTailTileContext(nc) as tc:
        with (
            tc.tile_pool(name="data", bufs=1) as data,
            tc.tile_pool(name="consts", bufs=1) as consts,
            tc.tile_pool(name="work", bufs=2) as work,
            tc.tile_pool(name="psg", bufs=1, space="PSUM") as psg,
            tc.tile_pool(name="psb", bufs=1, space="PSUM") as psb,
            tc.tile_pool(name="pst", bufs=1, space="PSUM") as pst,
        ):
            H = FREE // 2
            # u halves (1600B rows keep the DMA rings at full burst
            # efficiency): pairs 0-1 on the sync HWDGE ring, 2-3 on
            # scalar; the small const tensor follows on the sync ring
            # (a partition-split across both rings was tried and is a
            # wash: 64-partition transfers halve per-ring throughput)
            ubs = []
            cf = consts.tile([50, _CF_COLS], f32r)
            for h in range(2):
                ubh = data.tile([128, H], bf16, tag=f"ub{h}")
                eng = nc.sync if h == 0 else nc.scalar
                eng.dma_start(ubh[:], u_dram[:, h * H : (h + 1) * H])
                ubs.append(ubh)
            nc.sync.dma_start(cf[:], cf_dram)

            cff = cf[:].bitcast(f32)
            ident = cff[:, 0:50]
            mask = cff[:, 50:100]
            ones50r = cf[:, 100:150]
            ones_col = cff[:, 100:101]
            negident = cff[0:25, 150:175]
            b_m10 = cff[:, 175:176]
            b_zero = cff[:, 176:177]
            identB = ident.unsqueeze(1).broadcast_to([50, NB, 50])
            maskB = mask.unsqueeze(1).broadcast_to([50, NB, 50])
            negidentB = negident.unsqueeze(1).broadcast_to([25, NB, 25])

            # dummy activation on a memset scratch (no DMA deps): pulls the
            # single ACT table load to the head of the ACT queue, fully
            # hidden under the input DMAs
            tscr = work.tile([1, 1], f32, tag="tscr")
            nc.vector.memset(tscr[:], 1.0)
            nc.scalar.activation(tscr[:], tscr[:], Act.Exp, bias=tscr[:])

            # PE p-state warmup: sustained dummy matmuls during the input
            # DMA window so the real gram chains run at full clock
            wscr = work.tile([128, 400], bf16, tag="wscr")
            nc.vector.memset(wscr[:], 0.0)
            psw = pst.tile([50, 400], f32, tag="warm")
            for w in range(4):
                nc.tensor.matmul(
                    psw[:], wscr[:, 0:50], wscr[:],
                    start=(w == 0), stop=(w == 3),
                )

            # zero rows 25:50 of the -2*pos half of the final-sum tile
            fin = work.tile([50, 2 * NB], f32, tag="fin")
            nc.vector.memset(fin[:], 0.0)

            # 4 gram chains into one 50x200 PSUM tile
            gpall = psg.tile([50, 200], f32, tag="g")
            for b in range(NB):
                gslice = gpall[:, b * 50 : (b + 1) * 50]
                base = (b % 2) * 400
                for k in range(8):
                    sl = ubs[b // 2][:, base + k * 50 : base + (k + 1) * 50]
                    nc.tensor.matmul(gslice, sl, sl, start=(k == 0), stop=(k == 7))

            # block diagonals -> squared norms d [50,4]
            dmul = work.tile([50, 200], f32, tag="dmul")
            nc.vector.tensor_mul(blk(dmul[:]), blk(gpall[:]), identB)
            dsq = work.tile([50, NB], f32, tag="dsq")
            nc.vector.reduce_sum(dsq[:], blk(dmul[:]), axis=X)

            # inv = sqrt(10)*rsqrt(d) = exp(-0.5*ln(d/10)) on ACT
            lnd = work.tile([50, NB], f32, tag="lnd")
            nc.scalar.activation(lnd[:], dsq[:], Act.Ln, bias=b_zero, scale=0.1)
            inv = work.tile([50, NB], f32, tag="inv")
            nc.scalar.activation(inv[:], lnd[:], Act.Exp, bias=b_zero, scale=-0.5)

            # PSUM -> SBUF move doubling as the exact diagonal kill
            gsb = work.tile([50, 200], f32, tag="gsb")
            nc.vector.tensor_add(blk(gsb[:]), blk(gpall[:]), maskB)

            # colscale row: binv4[p,(b,c)] = inv[c,b] via ones^T @ diag(inv)
            invrep = inv[:].unsqueeze(2).broadcast_to([50, NB, 50])
            dinv = work.tile([50, 200], f32r, tag="dinv")
            nc.vector.tensor_mul(blk(dinv[:]), identB, invrep)
            binv4 = psb.tile([50, 200], f32, tag="binv4")
            nc.tensor.matmul(binv4[:], ones50r, dinv[:], start=True, stop=True)

            # S = (G+mask) * colscale * rowscale; eall = exp(S-10)
            gcol = work.tile([50, 200], f32, tag="gcol")
            nc.vector.tensor_mul(gcol[:], gsb[:], binv4[:])
            rowsc = work.tile([50, 200], f32, tag="rowsc")
            nc.vector.tensor_mul(blk(rowsc[:]), blk(gcol[:]), invrep)
            eall = work.tile([50, 200], f32, tag="eall")
            nc.scalar.activation(eall[:], rowsc[:], Act.Exp, bias=b_m10)
            esum = work.tile([50, NB], f32, tag="esum")
            nc.vector.reduce_sum(esum[:], blk(eall[:]), axis=X)

            # -2 * positives from the S12/S21 block diagonals of S
            pmul = work.tile([25, NB * 25], f32, tag="pmul")
            nc.vector.tensor_mul(
                blk(pmul[:], f=25),
                blk(rowsc[0:25, :])[:, :, 25:50],
                negidentB,
            )
            nc.vector.reduce_sum(fin[0:25, NB : 2 * NB], blk(pmul[:], f=25), axis=X)

            # lse-10 = ln(esum); one 50x8 reduce sums lse and -2*pos rows
            nc.scalar.activation(fin[:, 0:NB], esum[:], Act.Ln, bias=b_zero)
            acc = work.tile([50, 1], f32, tag="acc")
            nc.vector.reduce_sum(acc[:], fin[:], axis=X)

            # partition sum -> scalar; restore the +10 LSE shift (10*50*NB)
            # and apply 1/(2*R*B) in the same op
            tp = pst.tile([1, 1], f32, tag="tot")
            nc.tensor.matmul(tp[:], acc[:], ones_col, start=True, stop=True)
            scale = 1.0 / (2.0 * R * NB * NCORES)
            nc.vector.tensor_scalar(
                res_t.ap(), tp[:], scale, (10.0 * 50 * NB) * scale,
                op0=Alu.mult, op1=Alu.add,
            )

            # posted TENSOR_STORE of the scalar result through the
            # preloaded address pair: no output DMA round-trip, no pointer
            # fetch on the critical path, and the store flight is covered
            # by the tile drain barrier. (registers are untyped 32-bit —
            # move the f32 bits via an i32 view)
            nc.gpsimd.reg_load(rreg, res_t.ap().bitcast(i32))
            nc.gpsimd.store(opair, rreg)

    _regs.close()

    _strip_init_overhead(nc)
    nc.compile()
    return nc


def get_nc():
    global _nc_cache
    if _nc_cache is None:
        _nc_cache = _build_nc()
    return _nc_cache


def pack_inputs(proj: np.ndarray) -> np.ndarray:
    """(96,256,64,64) -> (128, 32, 8, 50) bf16: partition=c%128,
    free=(pair, chunk k=(cb,dy,dx), view, region rh*5+rw)."""
    import ml_dtypes

    win = np.array([[c - 1, c] for c in _CENTRES])  # (5, 2): rows/cols of each window
    v = np.stack([proj[32:64], proj[64:96]], axis=1)  # (32, 2, 256, 64, 64)
    g = v[:, :, :, win[:, :, None, None], win[None, None, :, :]]  # (32,2,256,5,2,5,2)
    g = g.reshape(32, 2, 2, 128, 5, 2, 5, 2)  # b, view, cb, c', rh, dy, rw, dx
    arr = np.transpose(g, (3, 0, 2, 5, 7, 1, 4, 6))  # c', b, cb, dy, dx, view, rh, rw
    return np.ascontiguousarray(arr).reshape(128, 32, 8, 50).astype(ml_dtypes.bfloat16)


def kernel(proj: np.ndarray) -> np.ndarray:
    from concourse.bass_utils import run_bass_kernel_spmd

    nc = get_nc()
    arr = pack_inputs(np.asarray(proj))
    cf = _build_consts()
    in_maps = [
        {
            "u": np.ascontiguousarray(arr[:, c * NB : (c + 1) * NB]).reshape(128, FREE),
            "cf": cf,
        }
        for c in range(NCORES)
    ]
    results = run_bass_kernel_spmd(nc, in_maps, list(range(NCORES))).results
    total = 0.0
    for r in results:
        total += float(r["out"][0, 0])
    return np.float32(total)
